# revision 1
# baseline (speedup 1.0000x reference)
"""Multi-head attention (B=4, S=2048, E=768, H=12) on 8 trn2 NeuronCores.

Sharding: 2-D (batch x head-half). Core c handles batch c//2, heads
(c%2)*6 .. (c%2)*6+5  (Wq/Wk/Wv column-split, Wo row-split). Each core
returns a partial O^T [768, S]; host sums the two head-halves per batch,
transposes, and adds the effective output bias (bo + bv@Wo — softmax rows
sum to 1, so V's bias contributes a constant row folded on the host).

Device kernel (per core), bf16 matmuls + fp32 PSUM:
  - masked keys are compacted away on host; padded keys get -30000 added
    via the exp's per-partition bias -> exp == 0.
  - scores/ctx computed transposed (S^T tiles [128 k, q]) so P^T feeds the
    context matmul directly; V carries an appended ones column so row 64
    of the context accumulator is the softmax denominator.
  - the two heads of a pair share one 2-bank PSUM tile [128, 2*QB], so a
    single wide Exp covers both (halves the Act instruction count).
  - normalization: reciprocal_approx_fast straight from the PSUM
    denominator row, then gpsimd partition_broadcast (exact, on the idle
    Pool engine) spreads it across 64 partitions; czT = ctx * bcast on DVE.
  - PE is kept back-logged through the Act-paced attention phase by a
    filler queue (normalization broadcasts + previous q-block's output
    projection), so the tensor engine never idles and holds its 2.4 GHz
    p-state; consecutive matmuls always target different PSUM banks.
"""

import os
import numpy as np
import ml_dtypes

E = 768
H = 12
D = 64
HALF = 384  # E // 2 output cols per head-half
N_CORES = 8

_CACHE = {}
_LAST = None  # last BassKernelResults (for test harness introspection)

bf16_np = ml_dtypes.bfloat16


def _build(S_q, S_pad):
    from collections import deque
    from contextlib import ExitStack
    import concourse.bass as bass
    import concourse.tile as tile
    from concourse import bacc, mybir

    bf16 = mybir.dt.bfloat16
    f32 = mybir.dt.float32
    FT = mybir.ActivationFunctionType

    NKC = S_pad // 128
    NMC = HALF // 128        # 3 proj-dim chunks (head pairs)
    NEC = E // 128           # 6 embed chunks
    QB = 512 if S_q % 512 == 0 else S_q
    NQB = S_q // QB

    def ntiles(total, step=512):
        return [(s, min(step, total - s)) for s in range(0, total, step)]

    nc = bacc.Bacc("TRN2", target_bir_lowering=False, debug=False,
                   num_devices=N_CORES)

    qT = nc.dram_tensor("qT", [E, S_q], bf16, kind="ExternalInput").ap()
    kT = nc.dram_tensor("kT", [E, S_pad], bf16, kind="ExternalInput").ap()
    vT = nc.dram_tensor("vT", [E, S_pad], bf16, kind="ExternalInput").ap()
    wq = nc.dram_tensor("wq", [E, HALF], bf16, kind="ExternalInput").ap()
    wk = nc.dram_tensor("wk", [E, HALF], bf16, kind="ExternalInput").ap()
    wv = nc.dram_tensor("wv", [E, HALF], bf16, kind="ExternalInput").ap()
    wo = nc.dram_tensor("wo", [HALF, E], bf16, kind="ExternalInput").ap()
    bq2 = nc.dram_tensor("bq2", [128, NMC], f32, kind="ExternalInput").ap()
    bk2 = nc.dram_tensor("bk2", [128, NMC], f32, kind="ExternalInput").ap()
    kbias = nc.dram_tensor("kbias", [128, NKC], f32, kind="ExternalInput").ap()
    oT = nc.dram_tensor("oT", [E, S_q], f32, kind="ExternalOutput").ap()

    with tile.TileContext(nc) as tc, ExitStack() as ctx:
        cons = ctx.enter_context(tc.tile_pool(name="cons", bufs=1))
        wp = ctx.enter_context(tc.tile_pool(name="wp", bufs=1))
        acts = ctx.enter_context(tc.tile_pool(name="acts", bufs=1))
        pp = ctx.enter_context(tc.tile_pool(name="pp", bufs=3))
        ost = ctx.enter_context(tc.tile_pool(name="ost", bufs=10))
        nrm = ctx.enter_context(tc.tile_pool(name="nrm", bufs=1))

        # ---- constant/small loads ----
        bq2_t = cons.tile([128, NMC], f32, tag="bq2")
        bk2_t = cons.tile([128, NMC], f32, tag="bk2")
        kb_t = cons.tile([128, NKC], f32, tag="kb")
        nc.sync.dma_start(bq2_t[:], bq2[:])
        nc.sync.dma_start(bk2_t[:], bk2[:])
        nc.sync.dma_start(kb_t[:], kbias[:])

        # ---- weight + input loads (inputs in a scoped pool, freed after proj)
        qkv = tc.tile_pool(name="qkv", bufs=1)
        inp = qkv.__enter__()
        wq_t = [wp.tile([128, HALF], bf16, tag=f"wq{e}", name=f"wq{e}") for e in range(NEC)]
        wk_t = [wp.tile([128, HALF], bf16, tag=f"wk{e}", name=f"wk{e}") for e in range(NEC)]
        wv_t = [wp.tile([128, HALF], bf16, tag=f"wv{e}", name=f"wv{e}") for e in range(NEC)]
        wo_t = [wp.tile([128, E], bf16, tag=f"wo{m}", name=f"wo{m}") for m in range(NMC)]
        kT_t = [inp.tile([128, S_pad], bf16, tag=f"kT{e}", name=f"kTt{e}") for e in range(NEC)]
        vT_t = [inp.tile([128, S_pad], bf16, tag=f"vT{e}", name=f"vTt{e}") for e in range(NEC)]
        qT_t = [inp.tile([128, S_q], bf16, tag=f"qT{e}", name=f"qTt{e}") for e in range(NEC)]
        for e in range(NEC):
            nc.sync.dma_start(wk_t[e][:], wk[128 * e:128 * (e + 1), :])
            nc.sync.dma_start(kT_t[e][:], kT[128 * e:128 * (e + 1), :])
        for e in range(NEC):
            nc.sync.dma_start(wv_t[e][:], wv[128 * e:128 * (e + 1), :])
            nc.sync.dma_start(vT_t[e][:], vT[128 * e:128 * (e + 1), :])
        for e in range(NEC):
            nc.sync.dma_start(wq_t[e][:], wq[128 * e:128 * (e + 1), :])
            nc.sync.dma_start(qT_t[e][:], qT[128 * e:128 * (e + 1), :])
        for m in range(NMC):
            nc.sync.dma_start(wo_t[m][:], wo[128 * m:128 * (m + 1), :])

        # ---- projections (pairs of output tiles -> alternating PSUM banks)
        kts = [acts.tile([128, S_pad], bf16, tag=f"kts{m}", name=f"kts{m}") for m in range(NMC)]
        qts = [acts.tile([128, S_q], bf16, tag=f"qts{m}", name=f"qts{m}") for m in range(NMC)]
        vhx = [acts.tile([128, 6, 128], bf16, tag=f"vhx{j}", name=f"vhx{j}") for j in range(NKC)]

        psp = tc.tile_pool(name="psp", bufs=1, space="PSUM")
        ps = psp.__enter__()

        def proj_kq(wt, xt, out, bias_t, total, ms=None):
            # out^T[m-chunk, n] accumulated over NEC embed chunks; n-tiles
            # processed in pairs so consecutive matmuls alternate banks.
            for m in (range(NMC) if ms is None else ms):
                tiles = ntiles(total)
                for i in range(0, len(tiles), 2):
                    pair = tiles[i:i + 2]
                    pjs = [ps.tile([128, 512], f32, tag=f"pj{j}", bufs=2,
                                   name=f"pj_{m}_{i}_{j}")
                           for j in range(len(pair))]
                    for e in range(NEC):
                        for j, (n0, nw) in enumerate(pair):
                            nc.tensor.matmul(
                                pjs[j][:, :nw],
                                wt[e][:, 128 * m:128 * (m + 1)],
                                xt[e][:, n0:n0 + nw],
                                start=(e == 0), stop=(e == NEC - 1))
                    for j, (n0, nw) in enumerate(pair):
                        nc.scalar.activation(out[m][:, n0:n0 + nw],
                                             pjs[j][:, :nw],
                                             FT.Identity,
                                             bias=bias_t[:, m:m + 1])

        proj_kq(wk_t, kT_t, kts, bk2_t, S_pad)

        # V projection: natural layout, s-chunk pairs
        for i in range(0, NKC, 2):
            js = [j for j in (i, i + 1) if j < NKC]
            pvs = [ps.tile([128, HALF], f32, tag=f"pv{j - i}", bufs=2,
                           name=f"pv{j}") for j in js]
            for e in range(NEC):
                for x, j in enumerate(js):
                    nc.tensor.matmul(pvs[x][:],
                                     vT_t[e][:, 128 * j:128 * (j + 1)],
                                     wv_t[e][:],
                                     start=(e == 0), stop=(e == NEC - 1))
            for x, j in enumerate(js):
                nc.vector.memset(vhx[j][:, :, 64:128], 1.0)
                nc.scalar.copy(vhx[j][:, :, 0:64],
                               pvs[x][:].rearrange("p (h d) -> p h d", h=6))

        proj_kq(wq_t, qT_t, qts, bq2_t, S_q, ms=(0, 1))
        psp.__exit__(None, None, None)

        # ---- attention ----
        czT = [acts.tile([128, S_q], bf16, tag=f"czT{m}", name=f"czT{m}") for m in range(NMC)]

        psa = tc.tile_pool(name="psa", bufs=1, space="PSUM")
        ps = psa.__enter__()

        fillq = deque()

        # allocate the C/fill PSUM tags before S2's first tile so S2 lands
        # on the banks vproj freed long ago rather than the ones the last
        # qproj pair is still evacuating at the phase transition
        ps.tile([128, QB], f32, tag="C", bufs=3, name="Cwarm")
        ps.tile([128, QB], f32, tag="fill", name="fillwarm")

        # qproj m=2 runs as pre-seeded fillers inside the attention phase
        # (needed only from head pair p=2, ~70 iterations in), so the Act
        # exp pipeline starts ~5us earlier and qb0 has PE backlog.
        def make_qprojm(m, n0, nw):
            def mk(e):
                def f():
                    if e == 0:
                        make_qprojm.pj = ps.tile([128, QB], f32, tag="fill",
                                                 name=f"qp{m}_{n0}")
                    pj = make_qprojm.pj
                    nc.tensor.matmul(pj[:, :nw],
                                     wq_t[e][:, 128 * m:128 * (m + 1)],
                                     qT_t[e][:, n0:n0 + nw],
                                     start=(e == 0), stop=(e == NEC - 1))
                    if e == NEC - 1:
                        nc.scalar.activation(qts[m][:, n0:n0 + nw],
                                             pj[:, :nw], FT.Identity,
                                             bias=bq2_t[:, m:m + 1])
                return f
            return [mk(e) for e in range(NEC)]

        for (n0, nw) in ntiles(S_q):
            fillq.extend(make_qprojm(2, n0, nw))

        def evac(u, C):
            # normalization runs entirely off the tensor engine. Partition
            # moves only work via DMA or partition_broadcast-from-partition-0
            # (cross-partition DVE reads silently corrupt), so: copy ctx+den
            # rows to SBUF, DMA the den row to partition 0, reciprocal there,
            # Pool-engine broadcast across 64 partitions, DVE multiply.
            qb, h = divmod(u, 6)
            m, half = divmod(h, 2)
            cs = nrm.tile([65, QB], f32, tag="cs", bufs=4, name=f"cs{u}")
            nc.vector.tensor_copy(cs[:], C[0:65, :])
            dnr = nrm.tile([1, QB], f32, tag="dnr", bufs=4, name=f"dnr{u}")
            nc.sync.dma_start(dnr[:], cs[64:65, :])
            rq = nrm.tile([1, QB], f32, tag="rq", bufs=4, name=f"rq{u}")
            nc.vector.reciprocal_approx_fast(rq[:], dnr[:])
            bcb = nrm.tile([64, QB], f32, tag="bcb", bufs=4, name=f"bcb{u}")
            nc.gpsimd.partition_broadcast(bcb[:], rq[:])
            nc.vector.tensor_mul(
                czT[m][64 * half:64 * (half + 1), qb * QB:(qb + 1) * QB],
                cs[0:64, :], bcb[:])

        def make_outproj(qb, ec):
            # 3 accumulating matmuls into the fill bank + evacuation
            t0 = qb * QB

            def mk(mm):
                def f():
                    if mm == 0:
                        make_outproj.po = ps.tile([128, QB], f32, tag="fill",
                                                  name=f"po{qb}_{ec}")
                    po = make_outproj.po
                    nc.tensor.matmul(po[:],
                                     wo_t[mm][:, 128 * ec:128 * (ec + 1)],
                                     czT[mm][:, t0:t0 + QB],
                                     start=(mm == 0), stop=(mm == NMC - 1))
                    if mm == NMC - 1:
                        ot = ost.tile([128, QB], f32, tag="ot",
                                      name=f"ot{qb}_{ec}")
                        nc.vector.tensor_copy(ot[:], po[:])
                        nc.sync.dma_start(
                            oT[128 * ec:128 * (ec + 1), t0:t0 + QB], ot[:])
                return f
            return [mk(mm) for mm in range(NMC)]

        for qb in range(NQB):
            q0 = qb * QB
            for p in range(NMC):  # head pair: hA=2p (rows 0-63), hB=2p+1
                if qb >= 1 and p == 1:
                    # enqueue one iteration late so the previous q-block's
                    # last normalization (DMA+recip+broadcast chain) lands
                    # before the first filler needs czT[2]
                    for ec in range(NEC):
                        fillq.extend(make_outproj(qb - 1, ec))
                hA, hB = 2 * p, 2 * p + 1
                # C tiles rotate through 3 banks so the next pair's first
                # context matmul never waits on the previous evacuation copy
                CA = ps.tile([128, QB], f32, tag="C", bufs=3,
                             name=f"CA{qb}_{p}")
                CB = ps.tile([128, QB], f32, tag="C", bufs=3,
                             name=f"CB{qb}_{p}")

                def sc2(kc, S2t):
                    nc.tensor.matmul(
                        S2t[:, 0:QB], kts[p][0:64, 128 * kc:128 * (kc + 1)],
                        qts[p][0:64, q0:q0 + QB],
                        start=True, stop=True, tile_position=(0, 0))
                    nc.tensor.matmul(
                        S2t[:, QB:2 * QB],
                        kts[p][64:128, 128 * kc:128 * (kc + 1)],
                        qts[p][64:128, q0:q0 + QB],
                        start=True, stop=True, tile_position=(64, 0))

                S2 = ps.tile([128, 2 * QB], f32, tag="S2", bufs=2,
                             name=f"S2_{qb}_{p}_0")
                sc2(0, S2)
                for kc in range(NKC):
                    S2n = None
                    if kc + 1 < NKC:
                        S2n = ps.tile([128, 2 * QB], f32, tag="S2", bufs=2,
                                      name=f"S2_{qb}_{p}_{kc + 1}")
                        sc2(kc + 1, S2n)
                    if fillq:
                        fillq.popleft()()
                    if len(fillq) > 8:
                        fillq.popleft()()
                    P2 = pp.tile([128, 2 * QB], bf16, tag="P2",
                                 name=f"P2_{qb}_{p}_{kc}")
                    nc.scalar.activation(P2[:], S2[:], FT.Exp,
                                         bias=kb_t[:, kc:kc + 1], scale=1.0)
                    nc.tensor.matmul(CA[:], vhx[kc][:, hA, :], P2[:, 0:QB],
                                     start=(kc == 0), stop=(kc == NKC - 1))
                    nc.tensor.matmul(CB[:], vhx[kc][:, hB, :],
                                     P2[:, QB:2 * QB],
                                     start=(kc == 0), stop=(kc == NKC - 1))
                    S2 = S2n
                evac(qb * 6 + hA, CA)
                evac(qb * 6 + hB, CB)

        # flush remaining output-projection fillers
        while fillq:
            fillq.popleft()()
        psa.__exit__(None, None, None)
        qkv.__exit__(None, None, None)

        # ---- output projection tail: last q-block. mm-major across 6 PSUM
        # banks so the czT[2]-dependent matmuls start 12 matmuls in, hiding
        # the last normalization chain's latency.
        pso = tc.tile_pool(name="pso", bufs=1, space="PSUM")
        ps = pso.__enter__()
        t0 = (NQB - 1) * QB
        pos = [ps.tile([128, QB], f32, tag=f"po{ec}", name=f"pot{ec}")
               for ec in range(NEC)]
        for mm in range(NMC):
            for ec in range(NEC):
                nc.tensor.matmul(pos[ec][:],
                                 wo_t[mm][:, 128 * ec:128 * (ec + 1)],
                                 czT[mm][:, t0:t0 + QB],
                                 start=(mm == 0), stop=(mm == NMC - 1))
        for ec in range(NEC):
            ot = ost.tile([128, QB], f32, tag="ot", name=f"ott{ec}")
            # split the tail evacuations across DVE and Act (both idle now),
            # and the final DMAs across both hwdge engines (SP + Act) so the
            # output-queue drain overlaps instead of serializing
            if ec % 2 == 0:
                nc.vector.tensor_copy(ot[:], pos[ec][:])
                nc.scalar.dma_start(oT[128 * ec:128 * (ec + 1), t0:t0 + QB],
                                    ot[:])
            else:
                nc.scalar.copy(ot[:], pos[ec][:])
                nc.sync.dma_start(oT[128 * ec:128 * (ec + 1), t0:t0 + QB],
                                  ot[:])
        pso.__exit__(None, None, None)

    nc.compile()
    return nc


def _numpy_fallback(q, k, v, mask, Wq, bq, Wk, bk, Wv, bv, Wo, bo):
    B, Sq, _ = q.shape
    qh = (q @ Wq + bq).reshape(B, Sq, H, D).transpose(0, 2, 1, 3)
    kh = (k @ Wk + bk).reshape(B, -1, H, D).transpose(0, 2, 1, 3)
    vh = (v @ Wv + bv).reshape(B, -1, H, D).transpose(0, 2, 1, 3)
    s = np.einsum("bhqd,bhkd->bhqk", qh, kh) / np.sqrt(np.float32(D))
    s = s + np.where(mask == 0, np.float32(-1e9), np.float32(0))[:, None, None, :]
    s = s - s.max(-1, keepdims=True)
    w = np.exp(s)
    w = w / w.sum(-1, keepdims=True)
    ctx = np.einsum("bhqk,bhkd->bqhd", w, vh).reshape(B, Sq, E)
    return (ctx @ Wo + bo).astype(np.float32)


def kernel(q, k, v, mask, Wq, bq, Wk, bk, Wv, bv, Wo, bo):
    global _LAST
    q = np.asarray(q, np.float32)
    k = np.asarray(k, np.float32)
    v = np.asarray(v, np.float32)
    mask = np.asarray(mask)
    Wq = np.asarray(Wq, np.float32)
    bq = np.asarray(bq, np.float32)
    Wk = np.asarray(Wk, np.float32)
    bk = np.asarray(bk, np.float32)
    Wv = np.asarray(Wv, np.float32)
    bv = np.asarray(bv, np.float32)
    Wo = np.asarray(Wo, np.float32)
    bo = np.asarray(bo, np.float32)

    B, S_q, _ = q.shape
    idxs = [np.flatnonzero(mask[b]) for b in range(B)]
    ns = [len(ix) for ix in idxs]
    if min(ns) == 0 or B * 2 != N_CORES or S_q % 512 != 0:
        return _numpy_fallback(q, k, v, mask, Wq, bq, Wk, bk, Wv, bv, Wo, bo)

    S_pad = max(128, ((max(ns) + 127) // 128) * 128)
    NKC = S_pad // 128
    NMC = HALF // 128

    key = (S_q, S_pad)
    if key not in _CACHE:
        _CACHE[key] = _build(S_q, S_pad)
    nc = _CACHE[key]

    scale = np.float32(1.0 / np.sqrt(D))
    in_maps = []
    for c in range(N_CORES):
        b, j = divmod(c, 2)
        cols = slice(j * HALF, (j + 1) * HALF)
        kc_ = np.zeros((S_pad, E), np.float32)
        kc_[:ns[b]] = k[b][idxs[b]]
        vc_ = np.zeros((S_pad, E), np.float32)
        vc_[:ns[b]] = v[b][idxs[b]]
        kb_vec = np.zeros(S_pad, np.float32)
        kb_vec[ns[b]:] = -30000.0
        in_maps.append({
            "qT": np.ascontiguousarray(q[b].T).astype(bf16_np),
            "kT": np.ascontiguousarray(kc_.T).astype(bf16_np),
            "vT": np.ascontiguousarray(vc_.T).astype(bf16_np),
            "wq": (Wq[:, cols] * scale).astype(bf16_np),
            "wk": np.ascontiguousarray(Wk[:, cols]).astype(bf16_np),
            "wv": np.ascontiguousarray(Wv[:, cols]).astype(bf16_np),
            "wo": np.ascontiguousarray(Wo[cols, :]).astype(bf16_np),
            "bq2": np.ascontiguousarray((bq[cols] * scale).reshape(NMC, 128).T),
            "bk2": np.ascontiguousarray(bk[cols].reshape(NMC, 128).T),
            "kbias": np.ascontiguousarray(kb_vec.reshape(NKC, 128).T),
        })

    from concourse.bass_utils import run_bass_kernel_spmd
    res = run_bass_kernel_spmd(nc, in_maps, list(range(N_CORES)))
    _LAST = res

    bo_eff = bo + bv @ Wo
    out = np.empty((B, S_q, E), np.float32)
    for b in range(B):
        out[b] = (res.results[2 * b]["oT"] + res.results[2 * b + 1]["oT"]).T
        out[b] += bo_eff
    return out



# revision 5
# speedup vs baseline: 1.0414x; 1.0414x over previous
"""Multi-head attention (B=4, S=2048, E=768, H=12) on 8 trn2 NeuronCores.

Sharding: 2-D (batch x head-half). Core c handles batch c//2, heads
(c%2)*6 .. (c%2)*6+5  (Wq/Wk/Wv column-split, Wo row-split). Each core
returns a partial O^T [768, S]; host sums the two head-halves per batch,
transposes, and adds the effective output bias (bo + bv@Wo — softmax rows
sum to 1, so V's bias contributes a constant row folded on the host).

Device kernel (per core), bf16 matmuls + fp32 PSUM:
  - masked keys are compacted away on host; padded keys get -30000 added
    via the exp's per-partition bias -> exp == 0.
  - scores/ctx computed transposed (S^T tiles [128 k, q]) so P^T feeds the
    context matmul directly; V carries an appended ones column so row 64
    of the context accumulator is the softmax denominator.
  - the two heads of a pair share one 2-bank PSUM tile [128, 2*QB], so a
    single wide Exp covers both (halves the Act instruction count).
  - normalization: reciprocal_approx_fast straight from the PSUM
    denominator row, then gpsimd partition_broadcast (exact, on the idle
    Pool engine) spreads it across 64 partitions; czT = ctx * bcast on DVE.
  - SOFTWARE PIPELINE ACROSS PHASES: only K-proj (m0,m1) and the first
    q-block of Q-proj m0 run before attention; the exp stream starts as
    soon as the first scores land (~20us instead of ~48us). V-proj,
    K-proj m2, the remaining Q-proj tiles and the output projections all
    run as PE filler work interleaved into the Act-paced attention loop.
    The first (qb0, p0) block defers its context matmuls behind a deep
    P2 buffer until the streamed V projection catches up.
  - DMA issue order is arrival order (single sync hwdge queue): k-side
    first, then the first q-block, then V (s-chunked), then the rest.
"""

import os
import numpy as np
import ml_dtypes

E = 768
H = 12
D = 64
HALF = 384  # E // 2 output cols per head-half
N_CORES = 8

_CACHE = {}
_LAST = None  # last BassKernelResults (for test harness introspection)

bf16_np = ml_dtypes.bfloat16


def _build(S_q, S_pad):
    from collections import deque
    from contextlib import ExitStack
    import concourse.bass as bass
    import concourse.tile as tile
    from concourse import bacc, mybir

    bf16 = mybir.dt.bfloat16
    f32 = mybir.dt.float32
    FT = mybir.ActivationFunctionType

    NKC = S_pad // 128
    NMC = HALF // 128        # 3 proj-dim chunks (head pairs)
    NEC = E // 128           # 6 embed chunks
    QB = 512 if S_q % 512 == 0 else S_q
    NQB = S_q // QB

    def ntiles(total, step=512):
        return [(s, min(step, total - s)) for s in range(0, total, step)]

    nc = bacc.Bacc("TRN2", target_bir_lowering=False, debug=False,
                   num_devices=N_CORES)

    qT = nc.dram_tensor("qT", [E, S_q], bf16, kind="ExternalInput").ap()
    kT = nc.dram_tensor("kT", [E, S_pad], bf16, kind="ExternalInput").ap()
    vT = nc.dram_tensor("vT", [E, S_pad], bf16, kind="ExternalInput").ap()
    wq = nc.dram_tensor("wq", [E, HALF], bf16, kind="ExternalInput").ap()
    wk = nc.dram_tensor("wk", [E, HALF], bf16, kind="ExternalInput").ap()
    wv = nc.dram_tensor("wv", [E, HALF], bf16, kind="ExternalInput").ap()
    wo = nc.dram_tensor("wo", [HALF, E], bf16, kind="ExternalInput").ap()
    bq2 = nc.dram_tensor("bq2", [128, NMC], f32, kind="ExternalInput").ap()
    bk2 = nc.dram_tensor("bk2", [128, NMC], f32, kind="ExternalInput").ap()
    kbias = nc.dram_tensor("kbias", [128, NKC], f32, kind="ExternalInput").ap()
    oT = nc.dram_tensor("oT", [E, S_q], f32, kind="ExternalOutput").ap()

    with tile.TileContext(nc) as tc, ExitStack() as ctx:
        cons = ctx.enter_context(tc.tile_pool(name="cons", bufs=1))
        wp = ctx.enter_context(tc.tile_pool(name="wp", bufs=1))
        acts = ctx.enter_context(tc.tile_pool(name="acts", bufs=1))
        pp = ctx.enter_context(tc.tile_pool(name="pp", bufs=8))
        ost = ctx.enter_context(tc.tile_pool(name="ost", bufs=6))
        nrm = ctx.enter_context(tc.tile_pool(name="nrm", bufs=1))

        # ---- constant/small loads ----
        bq2_t = cons.tile([128, NMC], f32, tag="bq2")
        bk2_t = cons.tile([128, NMC], f32, tag="bk2")
        kb_t = cons.tile([128, NKC], f32, tag="kb")
        nc.sync.dma_start(bq2_t[:], bq2[:])
        nc.sync.dma_start(bk2_t[:], bk2[:])
        nc.sync.dma_start(kb_t[:], kbias[:])

        # ---- weight + input tiles ----
        qkv = tc.tile_pool(name="qkv", bufs=1)
        inp = qkv.__enter__()
        wq_t = [wp.tile([128, HALF], bf16, tag=f"wq{e}", name=f"wq{e}") for e in range(NEC)]
        wk_t = [wp.tile([128, HALF], bf16, tag=f"wk{e}", name=f"wk{e}") for e in range(NEC)]
        wv_t = [wp.tile([128, HALF], bf16, tag=f"wv{e}", name=f"wv{e}") for e in range(NEC)]
        wo_t = [wp.tile([128, E], bf16, tag=f"wo{m}", name=f"wo{m}") for m in range(NMC)]
        kT_t = [inp.tile([128, S_pad], bf16, tag=f"kT{e}", name=f"kTt{e}") for e in range(NEC)]
        vT_t = [inp.tile([128, S_pad], bf16, tag=f"vT{e}", name=f"vTt{e}") for e in range(NEC)]
        qT_t = [inp.tile([128, S_q], bf16, tag=f"qT{e}", name=f"qTt{e}") for e in range(NEC)]

        # DMA issue order == arrival order on the single sync hwdge queue.
        # k-side first (gates the first scores), then first q-block, then
        # V in s-halves (consumed chunkwise by the streamed V-proj), then
        # the remaining q-blocks and Wo (needed only mid-window).
        VH1 = min(5 * 128, S_pad)
        for e in range(NEC):
            nc.sync.dma_start(wk_t[e][:], wk[128 * e:128 * (e + 1), :])
            nc.sync.dma_start(kT_t[e][:], kT[128 * e:128 * (e + 1), :])
        for e in range(NEC):
            nc.sync.dma_start(wq_t[e][:], wq[128 * e:128 * (e + 1), :])
        for e in range(NEC):
            nc.sync.dma_start(qT_t[e][:, 0:QB], qT[128 * e:128 * (e + 1), 0:QB])
        for e in range(NEC):
            nc.sync.dma_start(wv_t[e][:], wv[128 * e:128 * (e + 1), :])
        for e in range(NEC):
            nc.sync.dma_start(vT_t[e][:, 0:VH1], vT[128 * e:128 * (e + 1), 0:VH1])
        if VH1 < S_pad:
            for e in range(NEC):
                nc.sync.dma_start(vT_t[e][:, VH1:S_pad],
                                  vT[128 * e:128 * (e + 1), VH1:S_pad])
        for qb in range(1, NQB):
            q0 = qb * QB
            for e in range(NEC):
                nc.sync.dma_start(qT_t[e][:, q0:q0 + QB],
                                  qT[128 * e:128 * (e + 1), q0:q0 + QB])
        for m in range(NMC):
            nc.sync.dma_start(wo_t[m][:], wo[128 * m:128 * (m + 1), :])

        # ---- persistent activation tiles ----
        kts = [acts.tile([128, S_pad], bf16, tag=f"kts{m}", name=f"kts{m}") for m in range(NMC)]
        qts = [acts.tile([128, S_q], bf16, tag=f"qts{m}", name=f"qts{m}") for m in range(NMC)]
        vhx = [acts.tile([128, 6, 128], bf16, tag=f"vhx{j}", name=f"vhx{j}") for j in range(NKC)]
        czT = [acts.tile([128, S_q], bf16, tag=f"czT{m}", name=f"czT{m}") for m in range(NMC)]

        # ones columns of vhx (denominator rows) — DVE is idle now
        for j in range(NKC):
            nc.vector.memset(vhx[j][:, :, 64:128], 1.0)

        # ---- prefix projections: K-proj m0+m1, Q-proj m0 for qb0 ----
        psp = tc.tile_pool(name="psp", bufs=1, space="PSUM")
        ps = psp.__enter__()

        def proj_kq(wt, xt, out, bias_t, total, ms):
            for m in ms:
                tiles = ntiles(total)
                for i in range(0, len(tiles), 2):
                    pair = tiles[i:i + 2]
                    pjs = [ps.tile([128, 512], f32, tag=f"pj{j}", bufs=2,
                                   name=f"pj_{m}_{i}_{j}")
                           for j in range(len(pair))]
                    for e in range(NEC):
                        for j, (n0, nw) in enumerate(pair):
                            nc.tensor.matmul(
                                pjs[j][:, :nw],
                                wt[e][:, 128 * m:128 * (m + 1)],
                                xt[e][:, n0:n0 + nw],
                                start=(e == 0), stop=(e == NEC - 1))
                    for j, (n0, nw) in enumerate(pair):
                        nc.scalar.activation(out[m][:, n0:n0 + nw],
                                             pjs[j][:, :nw],
                                             FT.Identity,
                                             bias=bias_t[:, m:m + 1])

        proj_kq(wk_t, kT_t, kts, bk2_t, S_pad, ms=(0, 1))

        # Q-proj m0, first q-block only
        pq0 = ps.tile([128, QB], f32, tag="pj0", bufs=2, name="pq0")
        for e in range(NEC):
            nc.tensor.matmul(pq0[:], wq_t[e][:, 0:128], qT_t[e][:, 0:QB],
                             start=(e == 0), stop=(e == NEC - 1))
        nc.scalar.activation(qts[0][:, 0:QB], pq0[:], FT.Identity,
                             bias=bq2_t[:, 0:1])
        psp.__exit__(None, None, None)

        # ---- attention ----
        psa = tc.tile_pool(name="psa", bufs=1, space="PSUM")
        ps = psa.__enter__()

        # pin tag->bank ranges: C 2 banks, fill 2 banks, S2 2x2 banks
        ps.tile([128, QB], f32, tag="C", bufs=2, name="Cwarm")
        ps.tile([128, QB], f32, tag="fill", bufs=2, name="fillwarm")

        def sc2(p, q0, kc, S2t):
            nc.tensor.matmul(
                S2t[:, 0:QB], kts[p][0:64, 128 * kc:128 * (kc + 1)],
                qts[p][0:64, q0:q0 + QB],
                start=True, stop=True, tile_position=(0, 0))
            nc.tensor.matmul(
                S2t[:, QB:2 * QB],
                kts[p][64:128, 128 * kc:128 * (kc + 1)],
                qts[p][64:128, q0:q0 + QB],
                start=True, stop=True, tile_position=(64, 0))

        def evac(u, C, m, half):
            # normalization runs entirely off the tensor engine. Partition
            # moves only work via DMA or partition_broadcast-from-partition-0
            # (cross-partition DVE reads silently corrupt), so: copy ctx+den
            # rows to SBUF, DMA the den row to partition 0, reciprocal there,
            # Pool-engine broadcast across 64 partitions, DVE multiply.
            qb = u // 6
            cs = nrm.tile([65, QB], f32, tag="cs", bufs=4, name=f"cs{u}")
            nc.vector.tensor_copy(cs[:], C[0:65, :])
            dnr = nrm.tile([1, QB], f32, tag="dnr", bufs=4, name=f"dnr{u}")
            nc.sync.dma_start(dnr[:], cs[64:65, :])
            rq = nrm.tile([1, QB], f32, tag="rq", bufs=4, name=f"rq{u}")
            nc.vector.reciprocal_approx_fast(rq[:], dnr[:])
            bcb = nrm.tile([64, QB], f32, tag="bcb", bufs=4, name=f"bcb{u}")
            nc.gpsimd.partition_broadcast(bcb[:], rq[:])
            nc.vector.tensor_mul(
                czT[m][64 * half:64 * (half + 1), qb * QB:(qb + 1) * QB],
                cs[0:64, :], bcb[:])

        # ---------- filler generators (PE work inside the Act window) ----

        def kproj_unit(m, pair):
            # one K-proj n-tile pair for head-pair m, evac on Act (window
            # has Act slack early where these run)
            def f():
                pjs = [ps.tile([128, 512], f32, tag="fill", bufs=2,
                               name=f"kp{m}_{n0}") for n0, _ in pair]
                for e in range(NEC):
                    for j, (n0, nw) in enumerate(pair):
                        nc.tensor.matmul(
                            pjs[j][:, :nw],
                            wk_t[e][:, 128 * m:128 * (m + 1)],
                            kT_t[e][:, n0:n0 + nw],
                            start=(e == 0), stop=(e == NEC - 1))
                for j, (n0, nw) in enumerate(pair):
                    nc.scalar.activation(kts[m][:, n0:n0 + nw],
                                         pjs[j][:, :nw], FT.Identity,
                                         bias=bk2_t[:, m:m + 1])
            return f

        def vproj_unit(j0, j1):
            # V-proj for s-chunks j0 (and j1): natural layout, evac to vhx
            def f():
                js = [j for j in (j0, j1) if j is not None]
                pvs = [ps.tile([128, HALF], f32, tag="fill", bufs=2,
                               name=f"pv{j}") for j in js]
                for e in range(NEC):
                    for x, j in enumerate(js):
                        nc.tensor.matmul(pvs[x][:],
                                         vT_t[e][:, 128 * j:128 * (j + 1)],
                                         wv_t[e][:],
                                         start=(e == 0), stop=(e == NEC - 1))
                for x, j in enumerate(js):
                    nc.scalar.copy(vhx[j][:, :, 0:64],
                                   pvs[x][:].rearrange("p (h d) -> p h d", h=6))
            return f

        def make_qproj(m, qb):
            # Q-proj unit (6 matmul closures); evac on DVE so the exp
            # stream on Act is never interrupted mid-window
            cell = {}
            n0 = qb * QB

            def mk(e):
                def f():
                    if e == 0:
                        cell["pj"] = ps.tile([128, QB], f32, tag="fill",
                                             bufs=2, name=f"qp{m}_{qb}")
                    pj = cell["pj"]
                    nc.tensor.matmul(pj[:],
                                     wq_t[e][:, 128 * m:128 * (m + 1)],
                                     qT_t[e][:, n0:n0 + QB],
                                     start=(e == 0), stop=(e == NEC - 1))
                    if e == NEC - 1:
                        nc.vector.tensor_scalar_add(qts[m][:, n0:n0 + QB],
                                                    pj[:], bq2_t[:, m:m + 1])
                return f
            return [mk(e) for e in range(NEC)]

        def make_outproj(qb, ec):
            # 3 accumulating matmuls into a fill bank + evacuation
            t0 = qb * QB
            cell = {}

            def mk(mm):
                def f():
                    if mm == 0:
                        cell["po"] = ps.tile([128, QB], f32, tag="fill",
                                             bufs=2, name=f"po{qb}_{ec}")
                    po = cell["po"]
                    nc.tensor.matmul(po[:],
                                     wo_t[mm][:, 128 * ec:128 * (ec + 1)],
                                     czT[mm][:, t0:t0 + QB],
                                     start=(mm == 0), stop=(mm == NMC - 1))
                    if mm == NMC - 1:
                        ot = ost.tile([128, QB], f32, tag="ot",
                                      name=f"ot{qb}_{ec}")
                        nc.vector.tensor_copy(ot[:], po[:])
                        nc.sync.dma_start(
                            oT[128 * ec:128 * (ec + 1), t0:t0 + QB], ot[:])
                return f
            return [mk(mm) for mm in range(NMC)]

        # ---------- block B0 = (qb0, p0): custom loop with deferred ctx --

        p0_units = deque()
        ktiles = ntiles(S_pad)
        p0_units.append(kproj_unit(2, ktiles[0:2]))
        if len(ktiles) > 2:
            p0_units.append(kproj_unit(2, ktiles[2:]))
        qp10 = make_qproj(1, 0)
        def qp10_unit():
            for f in qp10:
                f()
        p0_units.append(qp10_unit)
        vps = [(j, j + 1 if j + 1 < NKC else None)
               for j in range(0, NKC, 2)]
        n_pre = len(p0_units)
        for j0, j1 in vps:
            p0_units.append(vproj_unit(j0, j1))

        CA = ps.tile([128, QB], f32, tag="C", bufs=2, name="CA0")
        CB = ps.tile([128, QB], f32, tag="C", bufs=2, name="CB0")
        S2 = ps.tile([128, 2 * QB], f32, tag="S2", bufs=2, name="S2_0_0")
        sc2(0, 0, 0, S2)
        P2s = {}
        vdone = 0       # vhx chunks whose proj has been issued
        ctx_next = 0    # next kc whose context matmuls to issue
        popped = 0

        def issue_ctx(kc):
            P2 = P2s.pop(kc)
            nc.tensor.matmul(CA[:], vhx[kc][:, 0, :], P2[:, 0:QB],
                             start=(kc == 0), stop=(kc == NKC - 1))
            nc.tensor.matmul(CB[:], vhx[kc][:, 1, :], P2[:, QB:2 * QB],
                             start=(kc == 0), stop=(kc == NKC - 1))

        for kc in range(NKC):
            S2n = None
            if kc + 1 < NKC:
                S2n = ps.tile([128, 2 * QB], f32, tag="S2", bufs=2,
                              name=f"S2_0_0_{kc + 1}")
                sc2(0, 0, kc + 1, S2n)
            P2 = pp.tile([128, 2 * QB], bf16, tag="P2", name=f"P2_0_0_{kc}")
            nc.scalar.activation(P2[:], S2[:], FT.Exp,
                                 bias=kb_t[:, kc:kc + 1], scale=1.0)
            P2s[kc] = P2
            if p0_units:
                p0_units.popleft()()
                popped += 1
                if popped > n_pre:
                    vdone = min(2 * (popped - n_pre), NKC)
            while ctx_next < min(vdone, kc + 1):
                issue_ctx(ctx_next)
                ctx_next += 1
            S2 = S2n
        while p0_units:
            p0_units.popleft()()
            popped += 1
            vdone = min(2 * (popped - n_pre), NKC)
        while ctx_next < NKC:
            issue_ctx(ctx_next)
            ctx_next += 1
        evac(0, CA, 0, 0)
        evac(1, CB, 0, 1)

        # ---------- remaining blocks: generic Act-paced loop ------------

        fillq = deque()
        blocks = [(qb, p) for qb in range(NQB) for p in range(NMC)]

        def enqueue_for(bi):
            # qproj(m, qb) must pop before block (qb, p=m); outproj(qb)
            # pops once czT[qb] is complete (one block of slack after
            # (qb, p2)'s normalization chains)
            qb, p = blocks[bi]
            if bi == 1:
                fillq.extend(make_qproj(2, 0))
                if NQB > 1:
                    fillq.extend(make_qproj(0, 1))
            elif bi == 2 and NQB > 1:
                fillq.extend(make_qproj(1, 1))
                fillq.extend(make_qproj(2, 1))
            elif p == 0 and qb >= 1 and qb + 1 < NQB:
                fillq.extend(make_qproj(0, qb + 1))
            elif p == 1 and qb >= 1:
                for ec in range(NEC):
                    fillq.extend(make_outproj(qb - 1, ec))
                if qb + 1 < NQB:
                    fillq.extend(make_qproj(1, qb + 1))
            elif p == 2 and qb >= 1 and qb + 1 < NQB:
                fillq.extend(make_qproj(2, qb + 1))

        for bi in range(1, len(blocks)):
            qb, p = blocks[bi]
            q0 = qb * QB
            enqueue_for(bi)
            hA, hB = 2 * p, 2 * p + 1
            CA = ps.tile([128, QB], f32, tag="C", bufs=2,
                         name=f"CA{qb}_{p}")
            CB = ps.tile([128, QB], f32, tag="C", bufs=2,
                         name=f"CB{qb}_{p}")
            S2 = ps.tile([128, 2 * QB], f32, tag="S2", bufs=2,
                         name=f"S2_{qb}_{p}_0")
            sc2(p, q0, 0, S2)
            for kc in range(NKC):
                S2n = None
                if kc + 1 < NKC:
                    S2n = ps.tile([128, 2 * QB], f32, tag="S2", bufs=2,
                                  name=f"S2_{qb}_{p}_{kc + 1}")
                    sc2(p, q0, kc + 1, S2n)
                if fillq:
                    fillq.popleft()()
                if len(fillq) > 6:
                    fillq.popleft()()
                P2 = pp.tile([128, 2 * QB], bf16, tag="P2",
                             name=f"P2_{qb}_{p}_{kc}")
                nc.scalar.activation(P2[:], S2[:], FT.Exp,
                                     bias=kb_t[:, kc:kc + 1], scale=1.0)
                nc.tensor.matmul(CA[:], vhx[kc][:, hA, :], P2[:, 0:QB],
                                 start=(kc == 0), stop=(kc == NKC - 1))
                nc.tensor.matmul(CB[:], vhx[kc][:, hB, :],
                                 P2[:, QB:2 * QB],
                                 start=(kc == 0), stop=(kc == NKC - 1))
                S2 = S2n
            evac(qb * 6 + hA, CA, p, 0)
            evac(qb * 6 + hB, CB, p, 1)

        # flush remaining output-projection fillers
        while fillq:
            fillq.popleft()()
        psa.__exit__(None, None, None)
        qkv.__exit__(None, None, None)

        # ---- output projection tail: last q-block. mm-major across 6 PSUM
        # banks so the czT[2]-dependent matmuls start 12 matmuls in, hiding
        # the last normalization chain's latency.
        pso = tc.tile_pool(name="pso", bufs=1, space="PSUM")
        ps = pso.__enter__()
        t0 = (NQB - 1) * QB
        pos = [ps.tile([128, QB], f32, tag=f"po{ec}", name=f"pot{ec}")
               for ec in range(NEC)]
        for mm in range(NMC):
            for ec in range(NEC):
                nc.tensor.matmul(pos[ec][:],
                                 wo_t[mm][:, 128 * ec:128 * (ec + 1)],
                                 czT[mm][:, t0:t0 + QB],
                                 start=(mm == 0), stop=(mm == NMC - 1))
        for ec in range(NEC):
            ot = ost.tile([128, QB], f32, tag="ot", name=f"ott{ec}")
            # split the tail evacuations across DVE and Act (both idle now),
            # and the final DMAs across both hwdge engines (SP + Act) so the
            # output-queue drain overlaps instead of serializing
            if ec % 2 == 0:
                nc.vector.tensor_copy(ot[:], pos[ec][:])
                nc.scalar.dma_start(oT[128 * ec:128 * (ec + 1), t0:t0 + QB],
                                    ot[:])
            else:
                nc.scalar.copy(ot[:], pos[ec][:])
                nc.sync.dma_start(oT[128 * ec:128 * (ec + 1), t0:t0 + QB],
                                  ot[:])
        pso.__exit__(None, None, None)

    nc.compile()
    return nc


def _numpy_fallback(q, k, v, mask, Wq, bq, Wk, bk, Wv, bv, Wo, bo):
    B, Sq, _ = q.shape
    qh = (q @ Wq + bq).reshape(B, Sq, H, D).transpose(0, 2, 1, 3)
    kh = (k @ Wk + bk).reshape(B, -1, H, D).transpose(0, 2, 1, 3)
    vh = (v @ Wv + bv).reshape(B, -1, H, D).transpose(0, 2, 1, 3)
    s = np.einsum("bhqd,bhkd->bhqk", qh, kh) / np.sqrt(np.float32(D))
    s = s + np.where(mask == 0, np.float32(-1e9), np.float32(0))[:, None, None, :]
    s = s - s.max(-1, keepdims=True)
    w = np.exp(s)
    w = w / w.sum(-1, keepdims=True)
    ctx = np.einsum("bhqk,bhkd->bqhd", w, vh).reshape(B, Sq, E)
    return (ctx @ Wo + bo).astype(np.float32)


def kernel(q, k, v, mask, Wq, bq, Wk, bk, Wv, bv, Wo, bo):
    global _LAST
    q = np.asarray(q, np.float32)
    k = np.asarray(k, np.float32)
    v = np.asarray(v, np.float32)
    mask = np.asarray(mask)
    Wq = np.asarray(Wq, np.float32)
    bq = np.asarray(bq, np.float32)
    Wk = np.asarray(Wk, np.float32)
    bk = np.asarray(bk, np.float32)
    Wv = np.asarray(Wv, np.float32)
    bv = np.asarray(bv, np.float32)
    Wo = np.asarray(Wo, np.float32)
    bo = np.asarray(bo, np.float32)

    B, S_q, _ = q.shape
    idxs = [np.flatnonzero(mask[b]) for b in range(B)]
    ns = [len(ix) for ix in idxs]
    if min(ns) == 0 or B * 2 != N_CORES or S_q % 512 != 0:
        return _numpy_fallback(q, k, v, mask, Wq, bq, Wk, bk, Wv, bv, Wo, bo)

    S_pad = max(256, ((max(ns) + 127) // 128) * 128)
    NKC = S_pad // 128
    NMC = HALF // 128

    key = (S_q, S_pad)
    if key not in _CACHE:
        _CACHE[key] = _build(S_q, S_pad)
    nc = _CACHE[key]

    scale = np.float32(1.0 / np.sqrt(D))
    in_maps = []
    for c in range(N_CORES):
        b, j = divmod(c, 2)
        cols = slice(j * HALF, (j + 1) * HALF)
        kc_ = np.zeros((S_pad, E), np.float32)
        kc_[:ns[b]] = k[b][idxs[b]]
        vc_ = np.zeros((S_pad, E), np.float32)
        vc_[:ns[b]] = v[b][idxs[b]]
        kb_vec = np.zeros(S_pad, np.float32)
        kb_vec[ns[b]:] = -30000.0
        in_maps.append({
            "qT": np.ascontiguousarray(q[b].T).astype(bf16_np),
            "kT": np.ascontiguousarray(kc_.T).astype(bf16_np),
            "vT": np.ascontiguousarray(vc_.T).astype(bf16_np),
            "wq": (Wq[:, cols] * scale).astype(bf16_np),
            "wk": np.ascontiguousarray(Wk[:, cols]).astype(bf16_np),
            "wv": np.ascontiguousarray(Wv[:, cols]).astype(bf16_np),
            "wo": np.ascontiguousarray(Wo[cols, :]).astype(bf16_np),
            "bq2": np.ascontiguousarray((bq[cols] * scale).reshape(NMC, 128).T),
            "bk2": np.ascontiguousarray(bk[cols].reshape(NMC, 128).T),
            "kbias": np.ascontiguousarray(kb_vec.reshape(NKC, 128).T),
        })

    from concourse.bass_utils import run_bass_kernel_spmd
    res = run_bass_kernel_spmd(nc, in_maps, list(range(N_CORES)))
    _LAST = res

    bo_eff = bo + bv @ Wo
    out = np.empty((B, S_q, E), np.float32)
    for b in range(B):
        out[b] = (res.results[2 * b]["oT"] + res.results[2 * b + 1]["oT"]).T
        out[b] += bo_eff
    return out
